# revision 1
# baseline (speedup 1.0000x reference)
"""Causal self-attention (B=2, T=2048, C=1024, H=16, rope) on 8 trn2 cores.

Sharding: core i = (batch b = i // 4, head-group g = i % 4 owning heads 4g..4g+3).
Each core computes its 4 heads' attention and a partial projection (transposed);
the host sums the 4 head-group partials per batch and adds b_proj.

On-core dataflow (all matmuls in float32r):
  xT = PE-transpose(x)                                   [C, T]
  qkT = W_qk_perm.T @ x.T   (+bias via K=1 matmul)       [512, T] -> RoPE on evict
  v   = x @ W_v             (+bias)                      [T, 256] natural, + ones col
  S.T = kT.T-slab @ qT-chunk (K=64)                      [128k, 512q] per tile
  expS = exp(S.T / 8)  (ACT), causal-masked on diagonal tiles
  [O.T; l] = sum_k [v | 1].T-slab @ expS                 PV + denominators in one go
  yT = O.T * (1/l)  (1/l broadcast via K=1 ones matmul)
  outT = W_p-shard.T @ yT                                [1024, T] partial, DMA out
"""

import os
import numpy as np

B, T, C, H = 2, 2048, 1024, 16
HS = C // H            # 64
HPC = H // 4           # 4 heads per core
NCORES = 8
TCH = 512              # t/q chunk size
NCH = T // TCH         # 4 chunks
NSLAB = T // 128       # 16 t-slabs

_cache = {}
last_results = None    # BassKernelResults of the most recent run (for test.py)


def _build():
    import concourse.bacc as bacc
    import concourse.mybir as mybir
    import concourse.tile as tile
    from concourse.masks import make_identity

    F32 = mybir.dt.float32
    F32R = mybir.dt.float32r
    AF = mybir.ActivationFunctionType

    nc = bacc.Bacc("TRN2", target_bir_lowering=False, debug=False,
                   num_devices=NCORES)

    x_in = nc.dram_tensor("x_in", (T, C), F32, kind="ExternalInput")
    wqk = nc.dram_tensor("wqk", (C, 512), F32, kind="ExternalInput")
    bqk = nc.dram_tensor("bqk", (1, 512), F32, kind="ExternalInput")
    wv = nc.dram_tensor("wv", (C, 256), F32, kind="ExternalInput")
    bv = nc.dram_tensor("bv", (1, 256), F32, kind="ExternalInput")
    wp = nc.dram_tensor("wp", (256, C), F32, kind="ExternalInput")
    cos_in = nc.dram_tensor("cos_in", (128, T), F32, kind="ExternalInput")
    sin_in = nc.dram_tensor("sin_in", (128, T), F32, kind="ExternalInput")
    cmask = nc.dram_tensor("cmask", (128, 4, TCH), F32, kind="ExternalInput")
    out_t = nc.dram_tensor("out_t", (C, T), F32, kind="ExternalOutput")

    with tile.TileContext(nc) as tc:
        with (
            tc.tile_pool(name="const", bufs=1) as const,
            tc.tile_pool(name="xp", bufs=2) as xp,
            tc.tile_pool(name="xtp", bufs=1) as xtp,
            tc.tile_pool(name="work", bufs=2) as work,
            tc.tile_pool(name="ep", bufs=4) as ep,
            tc.tile_pool(name="ost", bufs=2) as ost,
            tc.tile_pool(name="ps_a", bufs=2, space="PSUM") as ps_a,
            tc.tile_pool(name="ps_s", bufs=3, space="PSUM") as ps_s,
            tc.tile_pool(name="ps_o", bufs=2, space="PSUM") as ps_o,
            tc.tile_pool(name="ps_p", bufs=1, space="PSUM") as ps_p,
        ):
            # ---- constants / weights ----
            ident = const.tile([128, 128], F32)
            make_identity(nc, ident)

            ones_f = const.tile([1, TCH], F32)
            nc.gpsimd.memset(ones_f[:], 1.0)
            ones_r = const.tile([1, TCH], F32R)
            nc.vector.tensor_copy(ones_r[:], ones_f[:])

            wqk_sb = const.tile([128, 8, 512], F32R)
            for hf in range(2):
                stg = work.tile([128, 4, 512], F32, tag="stg")
                nc.sync.dma_start(
                    stg[:], wqk.ap()[hf * 512:(hf + 1) * 512, :]
                    .rearrange("(s p) m -> p s m", p=128))
                nc.vector.tensor_copy(wqk_sb[:, 4 * hf:4 * hf + 4, :], stg[:])
            bqk_sb = const.tile([1, 512], F32R)
            stgb = work.tile([1, 512], F32, tag="stgb")
            nc.sync.dma_start(stgb[:], bqk[:, :])
            nc.vector.tensor_copy(bqk_sb[:], stgb[:])

            wv_sb = const.tile([128, 8, 256], F32R)
            stg = work.tile([128, 8, 256], F32, tag="stg")
            nc.sync.dma_start(stg[:],
                              wv.ap().rearrange("(s p) m -> p s m", p=128))
            nc.vector.tensor_copy(wv_sb[:], stg[:])
            bv_sb = const.tile([1, 256], F32R)
            stgb = work.tile([1, 512], F32, tag="stgb")
            nc.sync.dma_start(stgb[0:1, 0:256], bv[:, :])
            nc.vector.tensor_copy(bv_sb[:], stgb[0:1, 0:256])

            wp_sb = const.tile([128, 2, C], F32R)
            stg = work.tile([128, 2, C], F32, tag="stg")
            nc.sync.dma_start(stg[:],
                              wp.ap().rearrange("(s p) m -> p s m", p=128))
            nc.vector.tensor_copy(wp_sb[:], stg[:])

            cos_sb = const.tile([128, T], F32)
            nc.sync.dma_start(cos_sb[:], cos_in[:, :])
            sin_sb = const.tile([128, T], F32)
            nc.sync.dma_start(sin_sb[:], sin_in[:, :])
            msk_sb = const.tile([128, 4, TCH], F32)
            nc.sync.dma_start(msk_sb[:], cmask[:, :, :])

            # ---- persistent activations ----
            qT = [const.tile([128, T], F32R, name=f"qT{p}", tag=f"qT{p}")
                  for p in range(2)]
            kT = [const.tile([128, T], F32R, name=f"kT{p}", tag=f"kT{p}")
                  for p in range(2)]
            # v with ones column: [t-slab-part, slab, head, 65]
            v_sb = const.tile([128, NSLAB, HPC, 65], F32R)
            ones128 = const.tile([128, 64], F32)
            nc.gpsimd.memset(ones128[:], 1.0)
            nc.vector.tensor_copy(
                v_sb[:, :, :, 64],
                ones128[:, 0:64].rearrange("p (s h) -> p s h", s=NSLAB))

            xT = [xtp.tile([128, TCH], F32R, name=f"xT{c}", tag=f"xT{c}")
                  for c in range(8)]

            for tcH in range(NCH):
                tcols = slice(tcH * TCH, (tcH + 1) * TCH)
                # ======== phase A: x load, transpose, qk(+rope), v ========
                for half in range(2):
                    xin = xp.tile([128, 2, C], F32, tag="xin")
                    r0 = tcH * TCH + half * 256
                    nc.sync.dma_start(
                        xin[:],
                        x_in[r0:r0 + 256, :].rearrange("(s p) m -> p s m", p=128))
                    for ts in range(2):
                        sl = half * 2 + ts  # slab within chunk, 0..3
                        for c in range(8):
                            pt = ps_a.tile([128, 128], F32, tag="a")
                            nc.tensor.transpose(
                                pt[:], xin[:, ts, c * 128:(c + 1) * 128], ident[:])
                            nc.vector.tensor_copy(
                                xT[c][:, sl * 128:(sl + 1) * 128], pt[:])

                # qk matmuls + rope eviction
                for m in range(4):
                    pqk = ps_a.tile([128, TCH], F32, tag="a")
                    for s in range(8):
                        nc.tensor.matmul(pqk[:], wqk_sb[:, s, m * 128:(m + 1) * 128],
                                         xT[s][:], start=(s == 0), stop=False)
                    nc.tensor.matmul(pqk[:], bqk_sb[0:1, m * 128:(m + 1) * 128],
                                     ones_r[0:1, :], start=False, stop=True,
                                     skip_group_check=True)
                    tQ = work.tile([128, TCH], F32, tag="tQ")
                    nc.scalar.activation(tQ[:], pqk[:], AF.Copy)
                    tA = work.tile([128, TCH], F32, tag="tA")
                    tB = work.tile([128, TCH], F32, tag="tB")
                    nc.vector.tensor_mul(tA[:], tQ[:], cos_sb[:, tcols])
                    dest = (qT if m % 2 == 0 else kT)[m // 2]
                    swap = [(0, 32, 32, 64), (32, 64, 0, 32),
                            (64, 96, 96, 128), (96, 128, 64, 96)]
                    for (a0, a1, b0, b1) in swap:
                        nc.vector.tensor_mul(tB[a0:a1, :], tQ[b0:b1, :],
                                             sin_sb[b0:b1, tcols])
                        nc.vector.tensor_add(dest[a0:a1, tcols],
                                             tA[a0:a1, :], tB[a0:a1, :])

                # v matmuls (natural orientation) + eviction into v_sb
                for ts in range(4):
                    pv = ps_a.tile([128, 256], F32, tag="a")
                    for s in range(8):
                        nc.tensor.matmul(pv[:], xT[s][:, ts * 128:(ts + 1) * 128],
                                         wv_sb[:, s, :], start=(s == 0), stop=False)
                    nc.tensor.matmul(pv[:], ones_r[0:1, 0:128], bv_sb[0:1, :],
                                     start=False, stop=True, skip_group_check=True)
                    sl = tcH * 4 + ts
                    nc.scalar.activation(
                        v_sb[:, sl, :, 0:64],
                        pv[:].rearrange("p (h e) -> p h e", e=64), AF.Copy)

                # ======== attention for q-chunk tcH ========
                yT_ch = work.tile([128, 2, TCH], F32R, tag="yT")
                nslabs = 4 * tcH + 4
                for p in range(2):
                    pos = [ps_o.tile([128, TCH], F32, tag="O", name=f"po{hh}")
                           for hh in range(2)]
                    for j in range(nslabs):
                        rr = j - 4 * tcH
                        r = max(rr, 0) * 128  # valid q-suffix start
                        qs = slice(tcH * TCH + r, (tcH + 1) * TCH)
                        for hh in range(2):
                            base, h, po = 64 * hh, 2 * p + hh, pos[hh]
                            psS = ps_s.tile([128, TCH], F32, tag="S")
                            nc.tensor.matmul(
                                psS[:, r:TCH],
                                kT[p][base:base + 64, j * 128:(j + 1) * 128],
                                qT[p][base:base + 64, qs],
                                start=True, stop=True)
                            expS = ep.tile([128, TCH], F32R, tag="expS")
                            nc.scalar.activation(expS[:, r:TCH], psS[:, r:TCH],
                                                 AF.Exp, scale=0.125)
                            if rr >= 0:
                                nc.gpsimd.tensor_mul(expS[:, r:r + 128],
                                        expS[:, r:r + 128],
                                        msk_sb[:, 0, 0:128])
                            nc.tensor.matmul(po[0:65, r:TCH], v_sb[:, j, h, :],
                                             expS[:, r:TCH],
                                             start=(j == 0),
                                             stop=(j == nslabs - 1))
                    for hh in range(2):
                        base, po = 64 * hh, pos[hh]
                        l_f = work.tile([1, TCH], F32, tag="lf")
                        nc.scalar.activation(l_f[:], po[64:65, :], AF.Copy)
                        l_r = work.tile([1, TCH], F32R, tag="lr")
                        with nc.allow_low_precision(reason="f32r rounding of 1/l"):
                            nc.vector.reciprocal(l_r[:], l_f[:])
                        prep = ps_p.tile([64, TCH], F32, tag="P")
                        nc.tensor.matmul(prep[:], ones_r[0:1, 0:64], l_r[0:1, :],
                                         start=True, stop=True)
                        nc.scalar.activation(yT_ch[base:base + 64, p, :],
                                             po[0:64, :], AF.Copy)
                        nc.vector.tensor_mul(yT_ch[base:base + 64, p, :],
                                             yT_ch[base:base + 64, p, :], prep[:])

                # ======== proj for q-chunk tcH ========
                for m in range(8):
                    pp = ps_p.tile([128, TCH], F32, tag="P")
                    for s in range(2):
                        nc.tensor.matmul(pp[:], wp_sb[:, s, m * 128:(m + 1) * 128],
                                         yT_ch[:, s, :], start=(s == 0),
                                         stop=(s == 1))
                    o_st = ost.tile([128, TCH], F32, tag="ost")
                    nc.scalar.activation(o_st[:], pp[:], AF.Copy)
                    nc.sync.dma_start(out_t[m * 128:(m + 1) * 128, tcols], o_st[:])

    nc.compile()
    return nc


def _rope_tables():
    pos = np.arange(T, dtype=np.float32)[:, None]                  # [T, 1]
    i = np.arange(1, HS // 2 + 1, dtype=np.float32)[None]          # [1, 32]
    theta = 1.0 / 10000.0 ** (2.0 * (i - 1.0) / HS)
    ang = pos * theta                                              # [T, 32]
    cos, sin = np.cos(ang).T, np.sin(ang).T                        # [32, T]
    cos_rep = np.tile(cos, (4, 1)).astype(np.float32)              # [128, T]
    sin_sgn = np.concatenate([sin, -sin, sin, -sin], 0).astype(np.float32)
    return cos_rep, sin_sgn


def _masks():
    p = np.arange(128)[:, None]
    f = np.arange(TCH)[None, :]
    m = np.stack([(p <= f - 128 * r) for r in range(4)], axis=1)   # [128, 4, TCH]
    return m.astype(np.float32)


def kernel(x, W_qkv, b_qkv, W_proj, b_proj):
    global last_results
    from concourse.bass_utils import run_bass_kernel_spmd

    if "nc" not in _cache:
        _cache["nc"] = _build()
    nc = _cache["nc"]

    x = np.asarray(x, np.float32)
    W_qkv = np.asarray(W_qkv, np.float32)
    b_qkv = np.asarray(b_qkv, np.float32)
    W_proj = np.asarray(W_proj, np.float32)
    b_proj = np.asarray(b_proj, np.float32)

    perm = np.concatenate([np.arange(0, HS, 2), np.arange(1, HS, 2)])  # even|odd
    cos_rep, sin_sgn = _rope_tables()
    cmask = _masks()

    in_maps = []
    for core in range(NCORES):
        b, g = core // 4, core % 4
        heads = [4 * g + j for j in range(HPC)]
        wq = [W_qkv[:, h * 3 * HS:h * 3 * HS + HS][:, perm] for h in heads]
        wk = [W_qkv[:, h * 3 * HS + HS:h * 3 * HS + 2 * HS][:, perm] for h in heads]
        wv = [W_qkv[:, h * 3 * HS + 2 * HS:h * 3 * HS + 3 * HS] for h in heads]
        bq = [b_qkv[h * 3 * HS:h * 3 * HS + HS][perm] for h in heads]
        bk = [b_qkv[h * 3 * HS + HS:h * 3 * HS + 2 * HS][perm] for h in heads]
        bv_ = [b_qkv[h * 3 * HS + 2 * HS:h * 3 * HS + 3 * HS] for h in heads]
        # col-chunks: [q01 | k01 | q23 | k23]
        wqk = np.concatenate([wq[0], wq[1], wk[0], wk[1],
                              wq[2], wq[3], wk[2], wk[3]], axis=1)
        bqk = np.concatenate([bq[0], bq[1], bk[0], bk[1],
                              bq[2], bq[3], bk[2], bk[3]])[None, :]
        in_maps.append({
            "x_in": np.ascontiguousarray(x[b]),
            "wqk": np.ascontiguousarray(wqk),
            "bqk": np.ascontiguousarray(bqk),
            "wv": np.ascontiguousarray(np.concatenate(wv, axis=1)),
            "bv": np.ascontiguousarray(np.concatenate(bv_)[None, :]),
            "wp": np.ascontiguousarray(W_proj[g * 256:(g + 1) * 256, :]),
            "cos_in": cos_rep,
            "sin_in": sin_sgn,
            "cmask": cmask,
        })

    res = run_bass_kernel_spmd(nc, in_maps, core_ids=list(range(NCORES)))
    last_results = res

    out = np.zeros((B, T, C), dtype=np.float32)
    for core in range(NCORES):
        b = core // 4
        out[b] += res.results[core]["out_t"].T
    out += b_proj[None, None, :]
    return out



# revision 33
# speedup vs baseline: 1.7288x; 1.7288x over previous
"""Causal self-attention (B=2, T=2048, C=1024, H=16, rope) on 8 trn2 cores.

Sharding: core i = (batch b = i // 4, head-group g = i % 4 owning heads 4g..4g+3).
Each core computes its 4 heads' attention and a partial projection (transposed);
the host sums the 4 head-group partials per batch and adds b_proj.

v3 design:
  - x shipped pre-transposed from host: xT bf16 [C, T] (and fp8e4 copy for the
    qk matmul) -> straight DMA into [128, 8, 512] SBUF tiles, no on-chip
    transpose at all.
  - qk projection matmul in fp8e4 with DoubleRow perf mode (0.5 cycles/row,
    K=256 per instruction); v projection and everything else bf16 (1 c/r).
  - RoPE partition-swap via a PE permutation matmul; elementwise muls on DVE
    at 2x bf16 rate; qk bias folded into the ACT eviction (bias AP).
  - softmax exp for a head PAIR in one ACT instruction ([128, 2, 512] PSUM
    pair tile) to amortize ACT per-instruction overhead.
  - epilogue: 1/l via DVE reciprocal straight from PSUM into a [2,512] tile,
    single K=2 matmul broadcasts both heads' 1/l, one DVE mul per head
    (PSUM x PSUM) producing bf16 yT; epilogue deferred into the next
    pair's S-window so the PE never waits on it.
  - phase A of chunk c+1 and proj of chunk c-1 are WOVEN into the attention
    instruction stream of chunk c; PV lags S by one slab.
  - outputs evicted bf16 (Pool engine) and summed on host in f32.
"""

import numpy as np

B, T, C, H = 2, 2048, 1024, 16
HS = C // H            # 64
HPC = H // 4           # 4 heads per core
NCORES = 8
TCH = 512              # t/q chunk size
NCH = T // TCH         # 4 chunks

FP8_QK = True          # qk projection matmul in fp8e4 DoubleRow

_cache = {}
last_results = None    # BassKernelResults of the most recent run (for test.py)


def _build():
    import concourse.bacc as bacc
    import concourse.mybir as mybir
    import concourse.tile as tile

    F32 = mybir.dt.float32
    F32R = mybir.dt.float32r
    BF16 = mybir.dt.bfloat16
    F8 = mybir.dt.float8e4
    AF = mybir.ActivationFunctionType
    DR = mybir.MatmulPerfMode.DoubleRow

    nc = bacc.Bacc("TRN2", target_bir_lowering=False, debug=False,
                   num_devices=NCORES)

    xt_in = nc.dram_tensor("xt_in", (128, 8, T), BF16, kind="ExternalInput")
    if FP8_QK:
        xt8_in = nc.dram_tensor("xt8_in", (128, 8, T), F8,
                                kind="ExternalInput")
        wqk = nc.dram_tensor("wqk", (128, 8, 512), F8, kind="ExternalInput")
    else:
        wqk = nc.dram_tensor("wqk", (128, 8, 512), BF16, kind="ExternalInput")
    bqk = nc.dram_tensor("bqk", (128, 4), F32, kind="ExternalInput")
    wv = nc.dram_tensor("wv", (128, 8, 256), BF16, kind="ExternalInput")
    bv = nc.dram_tensor("bv", (1, 256), BF16, kind="ExternalInput")
    wp = nc.dram_tensor("wp", (128, 2, C), BF16, kind="ExternalInput")
    cos_in = nc.dram_tensor("cos_in", (128, T), BF16, kind="ExternalInput")
    sin_in = nc.dram_tensor("sin_in", (128, T), BF16, kind="ExternalInput")
    perm_in = nc.dram_tensor("perm_in", (128, 128), BF16, kind="ExternalInput")
    mask_in = nc.dram_tensor("mask_in", (128, 2, 128), BF16,
                             kind="ExternalInput")
    out_t = nc.dram_tensor("out_t", (C, T), BF16, kind="ExternalOutput")

    with tile.TileContext(nc) as tc:
        with (
            tc.tile_pool(name="const", bufs=1) as const,
            tc.tile_pool(name="xtp", bufs=2) as xtp,
            tc.tile_pool(name="qsp", bufs=2) as qsp,
            tc.tile_pool(name="qnp", bufs=4) as qnp,
            tc.tile_pool(name="ep", bufs=3) as ep,
            tc.tile_pool(name="ytp", bufs=2) as ytp,
            tc.tile_pool(name="ost", bufs=4) as ost,
            tc.tile_pool(name="lrp", bufs=2) as lrp,
            tc.tile_pool(name="ps_s", bufs=2, space="PSUM") as ps_s,
            tc.tile_pool(name="ps_o", bufs=1, space="PSUM") as ps_o,
            tc.tile_pool(name="ps_a", bufs=2, space="PSUM") as ps_a,
        ):
            # ---- local constants (no DMA dependency) + PE warmup ----
            ones_b = const.tile([1, 512], BF16)
            nc.gpsimd.memset(ones_b[:], 1.0)
            ones64_f = const.tile([1, 64], F32)
            nc.gpsimd.memset(ones64_f[:], 1.0)
            ones64_r = const.tile([1, 64], F32R)
            nc.vector.tensor_copy(ones64_r[:], ones64_f[:])

            for w in range(16):
                pwu = ps_a.tile([128, TCH], F32, tag="a", name=f"wu{w}")
                nc.tensor.matmul(pwu[:], ones_b[0:1, 0:128], ones_b[0:1, :],
                                 start=True, stop=True)

            # ---- weights / tables (host-prepacked layouts, direct DMA) ----
            # order matters: DMA device serializes; earliest-needed first.
            wqk_sb = const.tile([128, 8, 512], F8 if FP8_QK else BF16)
            nc.sync.dma_start(wqk_sb[:], wqk[:, :, :])

            xT = [None] * NCH      # [128, 8, 512] bf16 per chunk (ring of 2)
            xT8 = [None] * NCH

            def dma_x8(c):
                if FP8_QK:
                    xt8 = xtp.tile([128, 8, TCH], F8, tag="xT8",
                                   name=f"xT8_{c}")
                    nc.sync.dma_start(xt8[:],
                                      xt8_in[:, :, c * TCH:(c + 1) * TCH])
                    xT8[c] = xt8

            def dma_xb(c):
                xt = xtp.tile([128, 8, TCH], BF16, tag="xT", name=f"xT_{c}")
                nc.sync.dma_start(xt[:], xt_in[:, :, c * TCH:(c + 1) * TCH])
                xT[c] = xt

            def dma_x(c):
                dma_x8(c)
                dma_xb(c)

            dma_x8(0)
            sin_sb = const.tile([128, T], BF16)
            nc.sync.dma_start(sin_sb[:, 0:TCH], sin_in[:, 0:TCH])
            cos_sb = const.tile([128, T], BF16)
            nc.sync.dma_start(cos_sb[:, 0:TCH], cos_in[:, 0:TCH])
            bqk_sb = const.tile([128, 4], F32)
            nc.sync.dma_start(bqk_sb[:], bqk[:, :])
            perm_sb = const.tile([128, 128], BF16)
            nc.sync.dma_start(perm_sb[:], perm_in[:, :])
            dma_xb(0)
            wv_sb = const.tile([128, 8, 256], BF16)
            nc.sync.dma_start(wv_sb[:], wv[:, :, :])
            bv_sb = const.tile([1, 256], BF16)
            nc.sync.dma_start(bv_sb[:], bv[:, :])
            msk_sb = const.tile([128, 2, 128], BF16)
            nc.sync.dma_start(msk_sb[:], mask_in[:, :, :])
            nc.sync.dma_start(sin_sb[:, TCH:T], sin_in[:, TCH:T])
            nc.sync.dma_start(cos_sb[:, TCH:T], cos_in[:, TCH:T])
            wp_sb = const.tile([128, 2, C], BF16)
            nc.sync.dma_start(wp_sb[:], wp[:, :, :])

            # persistent per-chunk activations
            qTc = [[const.tile([128, TCH], BF16, name=f"qT{p}_{c}",
                               tag=f"qT{p}_{c}") for c in range(NCH)]
                   for p in range(2)]
            kTc = [[const.tile([128, TCH], BF16, name=f"kT{p}_{c}",
                               tag=f"kT{p}_{c}") for c in range(NCH)]
                   for p in range(2)]
            # v natural [t-part, slab-in-chunk, head, 64 | ones]
            vc = [const.tile([128, 4, HPC, 65], BF16, name=f"v_{c}",
                             tag=f"v_{c}") for c in range(NCH)]
            for c in range(NCH):
                nc.gpsimd.memset(vc[c][:, :, :, 64], 1.0)

            # ---------- phase A items (qkv + rope for chunk c) ----------
            def qk_chain(c, m):
                # m: 0=q pair0, 1=k pair0, 2=q pair1, 3=k pair1
                def emit():
                    tcols = slice(c * TCH, (c + 1) * TCH)
                    pqk = ps_a.tile([128, TCH], F32, tag="a",
                                    name=f"pqk{c}_{m}")
                    if FP8_QK:
                        xt8 = xT8[c]
                        for s2 in range(4):
                            nc.tensor.matmul(
                                pqk[:],
                                wqk_sb[:, 2 * s2:2 * s2 + 2,
                                       m * 128:(m + 1) * 128],
                                xt8[:, 2 * s2:2 * s2 + 2, :],
                                start=(s2 == 0), stop=(s2 == 3),
                                perf_mode=DR)
                    else:
                        xt = xT[c]
                        for s in range(8):
                            nc.tensor.matmul(
                                pqk[:], wqk_sb[:, s, m * 128:(m + 1) * 128],
                                xt[:, s, :], start=(s == 0), stop=(s == 7))
                    qsb = qsp.tile([128, TCH], BF16, tag="qsb",
                                   name=f"qsb{c}_{m}")
                    nc.vector.tensor_scalar_add(qsb[:], pqk[:],
                                                bqk_sb[:, m:m + 1])
                    qsin = qnp.tile([128, TCH], BF16, tag="qn",
                                    name=f"qsin{c}_{m}")
                    nc.vector.tensor_mul(qsin[:], qsb[:], sin_sb[:, tcols])
                    d1 = qnp.tile([128, TCH], BF16, tag="qn",
                                  name=f"d1{c}_{m}")
                    nc.vector.tensor_mul(d1[:], qsb[:], cos_sb[:, tcols])
                    # reuse the pqk bank for the swap (pqk is consumed by the
                    # qsb eviction before this matmul's WAR dep resolves)
                    pswap = pqk
                    nc.tensor.matmul(pswap[:], perm_sb[:], qsin[:],
                                     start=True, stop=True,
                                     skip_group_check=True)
                    dest = (qTc if m % 2 == 0 else kTc)[m // 2][c]
                    nc.vector.tensor_add(dest[:, :], d1[:], pswap[:])
                return emit

            def v_item(c, ts):
                def emit():
                    xt = xT[c]
                    pv = ps_a.tile([128, 256], F32, tag="a",
                                   name=f"pv{c}_{ts}")
                    for s in range(8):
                        nc.tensor.matmul(pv[:],
                                         xt[:, s, ts * 128:(ts + 1) * 128],
                                         wv_sb[:, s, :],
                                         start=(s == 0), stop=False)
                    nc.tensor.matmul(pv[:], ones_b[0:1, 0:128], bv_sb[0:1, :],
                                     start=False, stop=True,
                                     skip_group_check=True)
                    nc.scalar.activation(
                        vc[c][:, ts, :, 0:64],
                        pv[:].rearrange("p (h e) -> p h e", e=64), AF.Copy)
                return emit

            def phase_a_items(c):
                items = []
                for m in range(4):
                    items.append(qk_chain(c, m))
                for ts in range(4):
                    items.append(v_item(c, ts))
                return items

            def proj_items(c):
                tcols = slice(c * TCH, (c + 1) * TCH)

                def proj_item(m):
                    def emit():
                        pp = ps_a.tile([128, TCH], F32, tag="a",
                                       name=f"pp{c}_{m}")
                        for s2 in range(2):
                            nc.tensor.matmul(
                                pp[:], wp_sb[:, s2, m * 128:(m + 1) * 128],
                                yT_ch[c][:, s2, :],
                                start=(s2 == 0), stop=(s2 == 1))
                        o_st = ost.tile([128, TCH], BF16, tag="ost",
                                        name=f"ost{c}_{m}")
                        if m % 2 == 0:
                            nc.vector.tensor_copy(o_st[:], pp[:])
                        else:
                            nc.scalar.activation(o_st[:], pp[:], AF.Copy)
                        nc.sync.dma_start(out_t[m * 128:(m + 1) * 128, tcols],
                                          o_st[:])
                    return emit
                return [proj_item(m) for m in range(8)]

            yT_ch = [None] * NCH

            # deferred epilogue: finish pair (c, p, pos) later, inside the
            # next pair's S-window
            def make_epilogue(c, p, pos):
                def emit():
                    for hh in range(2):
                        l_r = lrp.tile([1, TCH], F32R, tag="lr",
                                       name=f"lr{c}_{p}_{hh}")
                        with nc.allow_low_precision(reason="f32r 1/l"):
                            nc.vector.reciprocal(l_r[:], pos[hh][64:65, :])
                        prep = ps_a.tile([64, TCH], F32, tag="a",
                                         name=f"prep{c}_{p}_{hh}")
                        nc.tensor.matmul(prep[:], ones64_r[0:1, :],
                                         l_r[0:1, :], start=True, stop=True)
                        ysb = qnp.tile([64, TCH], BF16, tag="qn",
                                          name=f"ysb{c}_{p}_{hh}")
                        nc.scalar.activation(ysb[:], pos[hh][0:64, :],
                                             AF.Copy)
                        nc.vector.tensor_mul(
                            yT_ch[c][64 * hh:64 * hh + 64, p, :],
                            ysb[:], prep[:])
                return emit

            # ---------- prologue: chunk 0 phase A (nothing to weave into) ---
            for it in phase_a_items(0):
                it()

            pending_epi = None
            # ---------- main loop ----------
            for c in range(NCH):
                if c + 1 < NCH:
                    dma_x(c + 1)
                yT_ch[c] = ytp.tile([128, 2, TCH], BF16, tag="yT",
                                    name=f"yT_{c}")
                weave = []
                if c > 0:
                    weave += proj_items(c - 1)
                if c + 1 < NCH:
                    weave += phase_a_items(c + 1)
                nsl = 4 * c + 4
                nsteps = 2 * nsl
                wi = 0

                for p in range(2):
                    pos = [ps_o.tile([128, TCH], F32, tag=f"po{hh}",
                                     name=f"po{c}_{p}_{hh}")
                           for hh in range(2)]

                    def s_exp(j):
                        rr = j - 4 * c
                        r = max(rr, 0) * 128
                        jc, jt = j // 4, j % 4
                        psS = ps_s.tile([128, 2, TCH], F32, tag="S",
                                        name=f"S{c}_{p}_{j}")
                        for hh in range(2):
                            nc.tensor.matmul(
                                psS[:, hh, r:TCH],
                                kTc[p][jc][64 * hh:64 * hh + 64,
                                           jt * 128:(jt + 1) * 128],
                                qTc[p][c][64 * hh:64 * hh + 64, r:TCH],
                                start=True, stop=True)
                        expS = ep.tile([128, 2, TCH], BF16, tag="expS",
                                       name=f"e{c}_{p}_{j}")
                        nc.scalar.activation(expS[:, :, r:TCH],
                                             psS[:, :, r:TCH],
                                             AF.Exp, scale=0.125)
                        if rr >= 0:
                            # zero the future positions of the diagonal block
                            # (Pool is SBUF-only; this is its one job)
                            nc.gpsimd.tensor_mul(expS[:, :, r:r + 128],
                                                 expS[:, :, r:r + 128],
                                                 msk_sb[:, :, :])
                        return expS

                    def pv(j, expS):
                        rr = j - 4 * c
                        r = max(rr, 0) * 128
                        jc, jt = j // 4, j % 4
                        for hh in range(2):
                            nc.tensor.matmul(
                                pos[hh][0:65, r:TCH],
                                vc[jc][:, jt, 2 * p + hh, :],
                                expS[:, hh, r:TCH],
                                start=(j == 0), stop=(j == nsl - 1))

                    nonlocal_exp = [None] * nsl
                    nonlocal_exp[0] = s_exp(0)
                    nonlocal_exp[1] = s_exp(1)
                    if pending_epi is not None:
                        pending_epi()
                        pending_epi = None
                    nonlocal_exp[2] = s_exp(2)
                    for j in range(nsl):
                        if j + 3 < nsl:
                            nonlocal_exp[j + 3] = s_exp(j + 3)
                        pv(j, nonlocal_exp[j])
                        nonlocal_exp[j] = None
                        step = p * nsl + j + 1
                        want = (step * len(weave)) // nsteps if weave else 0
                        while wi < want:
                            weave[wi]()
                            wi += 1
                    pending_epi = make_epilogue(c, p, pos)
                while wi < len(weave):
                    weave[wi]()
                    wi += 1

            pending_epi()
            for it in proj_items(NCH - 1):
                it()

    nc.compile()
    return nc


def _rope_tables():
    pos = np.arange(T, dtype=np.float32)[:, None]                  # [T, 1]
    i = np.arange(1, HS // 2 + 1, dtype=np.float32)[None]          # [1, 32]
    theta = 1.0 / 10000.0 ** (2.0 * (i - 1.0) / HS)
    ang = pos * theta                                              # [T, 32]
    cos, sin = np.cos(ang).T, np.sin(ang).T                        # [32, T]
    cos_rep = np.tile(cos, (4, 1)).astype(np.float32)              # [128, T]
    sin_sgn = np.concatenate([sin, -sin, sin, -sin], 0).astype(np.float32)
    return cos_rep, sin_sgn


def _perm_matrix():
    # pswap[m] = qsin[sigma(m)]; P[k, m] = 1 iff k = sigma(m)
    sigma = np.arange(128)
    for a, b in ((0, 32), (64, 96)):
        sigma[a:a + 32], sigma[b:b + 32] = (
            np.arange(b, b + 32), np.arange(a, a + 32))
    P = np.zeros((128, 128), dtype=np.float32)
    P[sigma, np.arange(128)] = 1.0
    return P


def kernel(x, W_qkv, b_qkv, W_proj, b_proj):
    global last_results
    import ml_dtypes
    from concourse.bass_utils import run_bass_kernel_spmd

    BF = ml_dtypes.bfloat16
    F8 = ml_dtypes.float8_e4m3

    if "nc" not in _cache:
        _cache["nc"] = _build()
    nc = _cache["nc"]

    x = np.asarray(x, np.float32)
    W_qkv = np.asarray(W_qkv, np.float32)
    b_qkv = np.asarray(b_qkv, np.float32)
    W_proj = np.asarray(W_proj, np.float32)
    b_proj = np.asarray(b_proj, np.float32)

    perm = np.concatenate([np.arange(0, HS, 2), np.arange(1, HS, 2)])  # e|o
    cos_rep, sin_sgn = _rope_tables()
    P = _perm_matrix()
    # keep-mask: 1 where key partition p may attend to query column f
    pf = np.arange(128)[:, None] <= np.arange(128)[None, :]
    msk2 = np.repeat(pf[:, None, :], 2, axis=1).astype(np.float32)

    def shard(a, n=128):  # [(s p), m] -> [p, s, m]
        s = a.shape[0] // n
        return np.ascontiguousarray(
            a.reshape(s, n, *a.shape[1:]).transpose(1, 0, 2))

    in_maps = []
    for core in range(NCORES):
        b, g = core // 4, core % 4
        heads = [4 * g + j for j in range(HPC)]
        wq = [W_qkv[:, h * 3 * HS:h * 3 * HS + HS][:, perm] for h in heads]
        wk = [W_qkv[:, h * 3 * HS + HS:h * 3 * HS + 2 * HS][:, perm]
              for h in heads]
        wv_ = [W_qkv[:, h * 3 * HS + 2 * HS:h * 3 * HS + 3 * HS]
               for h in heads]
        bq = [b_qkv[h * 3 * HS:h * 3 * HS + HS][perm] for h in heads]
        bk = [b_qkv[h * 3 * HS + HS:h * 3 * HS + 2 * HS][perm] for h in heads]
        bv_ = [b_qkv[h * 3 * HS + 2 * HS:h * 3 * HS + 3 * HS] for h in heads]
        # col-chunks (m-tiles): [q01 | k01 | q23 | k23]
        wqk_full = np.concatenate([wq[0], wq[1], wk[0], wk[1],
                                   wq[2], wq[3], wk[2], wk[3]], axis=1)
        bqk_cols = np.stack([
            np.concatenate([bq[0], bq[1]]),
            np.concatenate([bk[0], bk[1]]),
            np.concatenate([bq[2], bq[3]]),
            np.concatenate([bk[2], bk[3]]),
        ], axis=1)                                                  # [128, 4]
        xt = shard(np.ascontiguousarray(x[b].T))                # [128, 8, T]
        im = {
            "xt_in": xt.astype(BF),
            "wqk": shard(wqk_full).astype(F8 if FP8_QK else BF),
            "bqk": np.ascontiguousarray(bqk_cols),
            "wv": shard(np.concatenate(wv_, axis=1)).astype(BF),
            "bv": np.concatenate(bv_)[None, :].astype(BF),
            "wp": shard(W_proj[g * 256:(g + 1) * 256, :]).astype(BF),
            "cos_in": cos_rep.astype(BF),
            "sin_in": sin_sgn.astype(BF),
            "perm_in": P.astype(BF),
            "mask_in": msk2.astype(BF),
        }
        if FP8_QK:
            im["xt8_in"] = xt.astype(F8)
        in_maps.append(im)

    res = run_bass_kernel_spmd(nc, in_maps, core_ids=list(range(NCORES)))
    last_results = res

    out = np.zeros((B, T, C), dtype=np.float32)
    for core in range(NCORES):
        b = core // 4
        out[b] += res.results[core]["out_t"].astype(np.float32).T
    out += b_proj[None, None, :]
    return out
